# revision 6
# baseline (speedup 1.0000x reference)
"""Multi-head attention (B=4, S=2048, D=1024, H=16, Dh=64) on 8 trn2 NeuronCores.

Sharding: core c -> heads (2c, 2c+1) of ALL 4 batches.  Every batch has 16
heads, so each core gets exactly 2 heads x 4 batches and per-core attention
work is Sum_b SQT_b*SKT_b score tiles -- perfectly balanced across cores
regardless of the per-batch sequence lengths (the old batch-sharded layout
made the largest-batch core ~2.1x slower than the mean).

Per core (2 heads, head A on partitions 0:64, head B on 64:128):
  - Host pre-transposes X per batch (D-major) in bf16 and concatenates the
    batches along seq: xq [D, SQtot], xk/xv [D, SKtot] (V rows >= V_len are
    zeroed on host).
  - Projections: qT/kT in [dh, seq] orientation, v in natural [seq, dh]
    orientation with a mask column appended per head (denominator trick).
  - QK computes scoresT[sk, sq] with K=64 contraction, the two heads issued
    back-to-back to complementary row groups (tile_position (0,0)/(64,0)) so
    they run concurrently in the PE array -- 2x QK throughput vs zero-padding
    the contraction to 128.
  - exp on ScalarE in groups of up to 3 sk-tiles (one 3-bank PSUM tile per
    group) to amortize the ~293ns fixed ACTIVATE overhead.
  - PV accumulates oT[65, sq] per head (row 64 = softmax denominator via the
    mask column).  NO on-device transpose or normalization: the kernel ships
    oT + denominator to DRAM and the HOST does o = (num/den).T and the
    Q_len row masking during unsharding.  This removes the fp32 PE-transpose
    matmuls (~80us of PE time in the old kernel) entirely.
  - Emission is software-pipelined: the next batch's projection pieces and
    the previous chunk's PV pieces are interleaved between QK groups to keep
    the in-order PE queue dense while ScalarE (the attention-phase
    bottleneck) drains the exp queue.

The program is compiled for the runtime tile counts (SQT_b, SKT_b) =
ceil(len/128) per batch (shared SPMD program across the 8 cores).
"""

import math

import numpy as np
import ml_dtypes


def _ensure_paths():
    import sys
    try:
        import concourse  # noqa: F401
        return
    except ImportError:
        pass
    for p in ("/opt/trn_rl_repo", "/root/.axon_site/_ro/trn_rl_repo"):
        if p not in sys.path:
            sys.path.insert(0, p)
    import concourse  # noqa: F401


P = 128          # SBUF partitions
D = 1024         # model dim
DH = 64          # head dim
KT = D // P      # contraction tiles for projections
GN = 3           # sk-tiles per exp group (3 PSUM banks)
NB = 4           # batches
NCORES = 8

_PROG_CACHE = {}

# exposed for test.py profiling reruns
_last_nc = None
_last_in_maps = None


def _chunks(total, sz=512):
    out = []
    o = 0
    while o < total:
        n = min(sz, total - o)
        out.append((o, n))
        o += n
    return out


def _build_program(SQT, SKT):
    """Build + bacc-compile the shared SPMD program for given per-batch tile
    counts (SQT, SKT are 4-tuples)."""
    _ensure_paths()
    import concourse.bass as bass  # noqa: F401
    import concourse.tile as tile
    from concourse import bacc, mybir

    BF = mybir.dt.bfloat16
    F32 = mybir.dt.float32
    Exp = mybir.ActivationFunctionType.Exp

    SQ = [t * P for t in SQT]
    SK = [t * P for t in SKT]
    SQtot = sum(SQ)
    SKtot = sum(SK)
    QOFF = [sum(SQ[:b]) for b in range(NB)]
    KOFF = [sum(SK[:b]) for b in range(NB)]
    TOFF = [sum(SKT[:b]) for b in range(NB)]
    SKTtot = sum(SKT)
    ATM = max(SKT)

    nc = bacc.Bacc("TRN2", target_bir_lowering=False, debug=False,
                   num_devices=NCORES)

    xq = nc.dram_tensor("xq", [D, SQtot], BF, kind="ExternalInput").ap()
    xk = nc.dram_tensor("xk", [D, SKtot], BF, kind="ExternalInput").ap()
    xv = nc.dram_tensor("xv", [D, SKtot], BF, kind="ExternalInput").ap()
    wq = nc.dram_tensor("wq", [D, P], BF, kind="ExternalInput").ap()
    wk = nc.dram_tensor("wk", [D, P], BF, kind="ExternalInput").ap()
    wv = nc.dram_tensor("wv", [D, P], BF, kind="ExternalInput").ap()
    mk2 = nc.dram_tensor("mk2", [SKtot, 2, 1], BF, kind="ExternalInput").ap()
    out = nc.dram_tensor("out", [2, DH + 1, SQtot], F32,
                         kind="ExternalOutput").ap()

    xq_r = xq.rearrange("(k p) s -> p k s", p=P)
    xk_r = xk.rearrange("(k p) s -> p k s", p=P)
    xv_r = xv.rearrange("(k p) s -> p k s", p=P)

    VW = 2 * (DH + 1)        # 130: [A num 64 | A mask | B num 64 | B mask]
    VWP = VW + DH - 1        # padded so lhsT for head B reads 128 cols

    with tile.TileContext(nc) as tc:
        with tc.tile_pool(name="const", bufs=1) as const, \
             tc.tile_pool(name="persist", bufs=1) as persist, \
             tc.tile_pool(name="xs", bufs=3) as xs, \
             tc.tile_pool(name="atp", bufs=2) as atp, \
             tc.tile_pool(name="otp", bufs=2) as otp, \
             tc.tile_pool(name="psq", bufs=2, space="PSUM") as psq, \
             tc.tile_pool(name="pss", bufs=2, space="PSUM") as pss:

            wq_sb = const.tile([P, KT, P], BF, tag="wq")
            wk_sb = const.tile([P, KT, P], BF, tag="wk")
            wv_sb = const.tile([P, KT, P], BF, tag="wv")
            nc.sync.dma_start(out=wq_sb, in_=wq.rearrange("(k p) e -> p k e", p=P))
            nc.sync.dma_start(out=wk_sb, in_=wk.rearrange("(k p) e -> p k e", p=P))
            nc.sync.dma_start(out=wv_sb, in_=wv.rearrange("(k p) e -> p k e", p=P))

            qt = persist.tile([P, SQtot], BF, tag="qt")
            kt = persist.tile([P, SKtot], BF, tag="kt")
            v_sb = persist.tile([P, SKTtot, VWP], BF, tag="v")
            # tail pad is read as lhsT columns for head B; zero it so
            # uninitialized SBUF never reaches PSUM
            nc.vector.memset(v_sb[:, :, VW:], 0.0)

            # ---------------- projection pieces (closures) ----------------
            def q_proj_piece(b, c0, n, dst, src_r, w_sb, off):
                def go():
                    xt = xs.tile([P, KT, 512], BF, tag="x")
                    nc.sync.dma_start(out=xt[:, :, :n],
                                      in_=src_r[:, :, off + c0:off + c0 + n])
                    ps = pss.tile([P, 512], F32, tag="acc")
                    for k in range(KT):
                        nc.tensor.matmul(ps[:, :n], w_sb[:, k, :], xt[:, k, :n],
                                         start=(k == 0), stop=(k == KT - 1))
                    nc.vector.tensor_copy(out=dst[:, off + c0:off + c0 + n],
                                          in_=ps[:, :n])
                return go

            def v_proj_piece(b, c0, n):
                def go():
                    xt = xs.tile([P, KT, 512], BF, tag="x")
                    nc.sync.dma_start(
                        out=xt[:, :, :n],
                        in_=xv_r[:, :, KOFF[b] + c0:KOFF[b] + c0 + n])
                    nt = n // P
                    ta = TOFF[b] + c0 // P
                    for st in range(nt):
                        ps = pss.tile([P, 512], F32, tag="acc")
                        for k in range(KT):
                            nc.tensor.matmul(
                                ps[:, :P], xt[:, k, P * st:P * (st + 1)],
                                wv_sb[:, k, :],
                                start=(k == 0), stop=(k == KT - 1))
                        vt = v_sb[:, ta + st, 0:VW].rearrange(
                            "p (g c) -> p g c", c=DH + 1)
                        nc.vector.tensor_copy(
                            out=vt[:, :, 0:DH],
                            in_=ps[:, :P].rearrange("p (g c) -> p g c", c=DH))
                        a0 = KOFF[b] + c0 + P * st
                        nc.sync.dma_start(out=vt[:, :, DH:DH + 1],
                                          in_=mk2[a0:a0 + P])
                return go

            def q_pieces(b):
                return [q_proj_piece(b, c0, n, qt, xq_r, wq_sb, QOFF[b])
                        for c0, n in _chunks(SQ[b])]

            def prelude(b):
                """Everything batch b's first attention chunk needs: kT, v,
                and the first q chunk."""
                ps_ = []
                for c0, n in _chunks(SK[b]):
                    ps_.append(q_proj_piece(b, c0, n, kt, xk_r, wk_sb, KOFF[b]))
                for c0, n in _chunks(SK[b]):
                    ps_.append(v_proj_piece(b, c0, n))
                ps_.append(q_pieces(b)[0])
                return ps_

            # ---------------- attention ----------------
            def pv_pieces(b, c0, n, ats):
                """PV + evac + output-DMA closures for one finished chunk."""
                pieces = []

                def mk_pv(g, t0, t1, po_box):
                    def go():
                        if t0 == 0:
                            po_box[0] = pss.tile([P, 512], F32, tag="acc",
                                                 name=f"po_{b}_{c0}_{g}")
                        po = po_box[0]
                        for t in range(t0, t1):
                            nc.tensor.matmul(
                                po[:, :n],
                                v_sb[:, TOFF[b] + t, (DH + 1) * g:
                                     (DH + 1) * g + P],
                                ats[g][:, t, :n],
                                start=(t == 0), stop=(t == SKT[b] - 1))
                        if t1 == SKT[b]:
                            ot = otp.tile([DH + 1, 512], F32, tag="ot",
                                          name=f"ot_{b}_{c0}_{g}")
                            nc.vector.tensor_copy(out=ot[:, :n],
                                                  in_=po[0:DH + 1, :n])
                            nc.sync.dma_start(
                                out=out[g, :, QOFF[b] + c0:QOFF[b] + c0 + n],
                                in_=ot[:, :n])
                    return go

                for g in range(2):
                    box = [None]
                    if SKT[b] > 6:
                        half = (SKT[b] + 1) // 2
                        pieces.append(mk_pv(g, 0, half, box))
                        pieces.append(mk_pv(g, half, SKT[b], box))
                    else:
                        pieces.append(mk_pv(g, 0, SKT[b], box))
                return pieces

            pend = []          # prev-chunk PV pieces (must emit next chunk)

            def emit_chunk(b, c0, n, fresh):
                """Emit one attention chunk; `fresh` (this batch's next q
                chunk + next batch's prelude ration) and the previous
                chunk's PV pieces are interleaved between QK groups."""
                nonlocal pend
                ats = (atp.tile([P, ATM, 512], BF, tag="ata",
                                name=f"ata_{b}_{c0}"),
                       atp.tile([P, ATM, 512], BF, tag="atb",
                                name=f"atb_{b}_{c0}"))
                groups = [(t0, min(GN, SKT[b] - t0))
                          for t0 in range(0, SKT[b], GN)]
                side = fresh + pend
                pend = []
                L = len(side)
                done = 0
                for gi, (t0, gn) in enumerate(groups):
                    pq = [psq.tile([P, GN, 512], F32, tag="qk",
                                   name=f"qk_{b}_{c0}_{t0}_{g}")
                          for g in range(2)]
                    for j in range(gn):
                        t = t0 + j
                        for g in range(2):
                            nc.tensor.matmul(
                                pq[g][:, j, :n],
                                kt[DH * g:DH * (g + 1),
                                   KOFF[b] + P * t:KOFF[b] + P * (t + 1)],
                                qt[DH * g:DH * (g + 1),
                                   QOFF[b] + c0:QOFF[b] + c0 + n],
                                start=True, stop=True)
                    for g in range(2):
                        nc.scalar.activation(
                            out=ats[g][:, t0:t0 + gn, :n],
                            in_=pq[g][:, 0:gn, :n],
                            func=Exp, scale=0.125)
                    upto = (L * (gi + 1)) // len(groups)
                    while done < upto:
                        side[done]()
                        done += 1
                while done < L:
                    side[done]()
                    done += 1
                pend = pv_pieces(b, c0, n, ats)

            # process batches smallest-first so attention (and ScalarE)
            # starts as soon as possible while the bigger batches' input
            # DMA streams in behind it
            BORD = sorted(range(NB), key=lambda b: SQT[b] * SKT[b])

            for piece in prelude(BORD[0]):
                piece()
            for bi, b in enumerate(BORD):
                nxt = BORD[bi + 1] if bi + 1 < NB else None
                filler = prelude(nxt) if nxt is not None else []
                qr = q_pieces(b)[1:]
                ch = _chunks(SQ[b])
                fdone = 0
                for ci, (c0, n) in enumerate(ch):
                    take = (len(filler) * (ci + 1)) // len(ch) - fdone
                    fresh = ([qr[ci]] if ci < len(qr) else []) \
                        + filler[fdone:fdone + take]
                    fdone += take
                    emit_chunk(b, c0, n, fresh)
            for piece in pend:
                piece()

    nc.compile()
    return nc


def _get_program(SQT, SKT):
    key = (tuple(SQT), tuple(SKT))
    if key not in _PROG_CACHE:
        _PROG_CACHE[key] = _build_program(key[0], key[1])
    return _PROG_CACHE[key]


def _prep_inputs(Q_seq, K_seq, V_seq, WQ, WK, WV, Q_len, V_len):
    """Host-side shared prep: per-batch transposed bf16 activations and
    masks, concatenated along seq; returns (SQT, SKT, shared dict)."""
    BF = ml_dtypes.bfloat16
    B = Q_seq.shape[0]
    SQT = [max(1, math.ceil(int(Q_len[b]) / P)) for b in range(B)]
    SKT = [max(1, math.ceil(int(V_len[b]) / P)) for b in range(B)]
    SQ = [t * P for t in SQT]
    SK = [t * P for t in SKT]

    xq = np.concatenate(
        [np.ascontiguousarray(Q_seq[b, :SQ[b]].T) for b in range(B)],
        axis=1).astype(BF)
    xk = np.concatenate(
        [np.ascontiguousarray(K_seq[b, :SK[b]].T) for b in range(B)],
        axis=1).astype(BF)
    mks = [(np.arange(SK[b]) < int(V_len[b])) for b in range(B)]
    xv = np.concatenate(
        [np.ascontiguousarray((V_seq[b, :SK[b]] * mks[b][:, None]).T)
         for b in range(B)], axis=1).astype(BF)
    mk2 = np.concatenate(mks)[:, None, None].astype(BF)
    mk2 = np.repeat(mk2, 2, axis=1)
    return SQT, SKT, {"xq": xq, "xk": xk, "xv": xv, "mk2": mk2}


def kernel(Q_seq, K_seq, V_seq, WQ, WK, WV, Q_len, V_len):
    global _last_nc, _last_in_maps
    _ensure_paths()
    from concourse.bass_utils import run_bass_kernel_spmd

    Q_seq = np.asarray(Q_seq, dtype=np.float32)
    K_seq = np.asarray(K_seq, dtype=np.float32)
    V_seq = np.asarray(V_seq, dtype=np.float32)
    WQ = np.asarray(WQ, dtype=np.float32)
    WK = np.asarray(WK, dtype=np.float32)
    WV = np.asarray(WV, dtype=np.float32)
    Q_len = np.asarray(Q_len).reshape(-1)
    V_len = np.asarray(V_len).reshape(-1)

    B, S, _ = Q_seq.shape
    BF = ml_dtypes.bfloat16

    SQT, SKT, shared = _prep_inputs(Q_seq, K_seq, V_seq, WQ, WK, WV,
                                    Q_len, V_len)
    SQ = [t * P for t in SQT]
    QOFF = [sum(SQ[:b]) for b in range(B)]

    nc = _get_program(SQT, SKT)

    in_maps = []
    for c in range(NCORES):
        m = dict(shared)
        m["wq"] = np.ascontiguousarray(WQ[:, P * c:P * (c + 1)]).astype(BF)
        m["wk"] = np.ascontiguousarray(WK[:, P * c:P * (c + 1)]).astype(BF)
        m["wv"] = np.ascontiguousarray(WV[:, P * c:P * (c + 1)]).astype(BF)
        in_maps.append(m)

    res = run_bass_kernel_spmd(nc, in_maps, core_ids=list(range(NCORES)))
    _last_nc, _last_in_maps = nc, in_maps

    H = 16
    full = np.zeros((B, S, H * DH), dtype=np.float32)
    for c in range(NCORES):
        o = res.results[c]["out"]          # [2, 65, SQtot]
        for g in range(2):
            h = 2 * c + g
            num = o[g, :DH]                # [64, SQtot]
            den = o[g, DH:DH + 1]          # [1, SQtot]
            ot = num / den
            for b in range(B):
                ql = int(Q_len[b])
                sl = ot[:, QOFF[b]:QOFF[b] + SQ[b]]
                full[b, :SQ[b], DH * h:DH * (h + 1)] = sl.T
                full[b, ql:, DH * h:DH * (h + 1)] = 0.0
    return full


# revision 11
# speedup vs baseline: 1.2449x; 1.2449x over previous
"""Multi-head attention (B=4, S=2048, D=1024, H=16, Dh=64) on 8 trn2 NeuronCores.

Sharding: core c -> heads (2c, 2c+1) of ALL 4 batches.  Every batch has 16
heads, so each core gets exactly 2 heads x 4 batches and per-core attention
work is Sum_b SQT_b*SKT_b score tiles -- perfectly balanced across cores
regardless of the per-batch sequence lengths (the old batch-sharded layout
made the largest-batch core ~2.1x slower than the mean).

Per core (2 heads, head A on partitions 0:64, head B on 64:128):
  - Host pre-transposes X per batch (D-major) in bf16 and concatenates the
    batches along seq: xq [D, SQtot], xk/xv [D, SKtot] (V rows >= V_len are
    zeroed on host).
  - Projections: qT/kT in [dh, seq] orientation, v in natural [seq, dh]
    orientation with a mask column appended per head (denominator trick).
  - QK computes scoresT[sk, sq] with K=64 contraction, the two heads issued
    back-to-back to complementary row groups (tile_position (0,0)/(64,0)) so
    they run concurrently in the PE array -- 2x QK throughput vs zero-padding
    the contraction to 128.
  - exp on ScalarE in groups of up to 3 sk-tiles (one 3-bank PSUM tile per
    group) to amortize the ~293ns fixed ACTIVATE overhead.
  - PV accumulates oT[65, sq] per head (row 64 = softmax denominator via the
    mask column).  NO on-device transpose or normalization: the kernel ships
    oT + denominator to DRAM and the HOST does o = (num/den).T and the
    Q_len row masking during unsharding.  This removes the fp32 PE-transpose
    matmuls (~80us of PE time in the old kernel) entirely.
  - Emission is software-pipelined: the next batch's projection pieces and
    the previous chunk's PV pieces are interleaved between QK groups to keep
    the in-order PE queue dense while ScalarE (the attention-phase
    bottleneck) drains the exp queue.

The program is compiled for the runtime tile counts (SQT_b, SKT_b) =
ceil(len/128) per batch (shared SPMD program across the 8 cores).
"""

import math

import numpy as np
import ml_dtypes


def _ensure_paths():
    import sys
    try:
        import concourse  # noqa: F401
        return
    except ImportError:
        pass
    for p in ("/opt/trn_rl_repo", "/root/.axon_site/_ro/trn_rl_repo"):
        if p not in sys.path:
            sys.path.insert(0, p)
    import concourse  # noqa: F401


P = 128          # SBUF partitions
D = 1024         # model dim
DH = 64          # head dim
KT = D // P      # contraction tiles for projections
GN = 3           # sk-tiles per exp group (3 PSUM banks)
NB = 4           # batches
NCORES = 8

_PROG_CACHE = {}

# exposed for test.py profiling reruns
_last_nc = None
_last_in_maps = None


def _chunks(total, sz=512):
    out = []
    o = 0
    while o < total:
        n = min(sz, total - o)
        out.append((o, n))
        o += n
    return out


def _build_program(SQT, SKT):
    """Build + bacc-compile the shared SPMD program for given per-batch tile
    counts (SQT, SKT are 4-tuples)."""
    _ensure_paths()
    import concourse.bass as bass  # noqa: F401
    import concourse.tile as tile
    from concourse import bacc, mybir

    BF = mybir.dt.bfloat16
    F32 = mybir.dt.float32
    Exp = mybir.ActivationFunctionType.Exp

    SQ = [t * P for t in SQT]
    SK = [t * P for t in SKT]
    SQtot = sum(SQ)
    SKtot = sum(SK)
    QOFF = [sum(SQ[:b]) for b in range(NB)]
    KOFF = [sum(SK[:b]) for b in range(NB)]
    TOFF = [sum(SKT[:b]) for b in range(NB)]
    SKTtot = sum(SKT)
    ATM = max(SKT)

    nc = bacc.Bacc("TRN2", target_bir_lowering=False, debug=False,
                   num_devices=NCORES)

    xq = nc.dram_tensor("xq", [D, SQtot], BF, kind="ExternalInput").ap()
    xk = nc.dram_tensor("xk", [D, SKtot], BF, kind="ExternalInput").ap()
    xv = nc.dram_tensor("xv", [D, SKtot], BF, kind="ExternalInput").ap()
    wq = nc.dram_tensor("wq", [D, P], BF, kind="ExternalInput").ap()
    wk = nc.dram_tensor("wk", [D, P], BF, kind="ExternalInput").ap()
    wv = nc.dram_tensor("wv", [D, P], BF, kind="ExternalInput").ap()
    mk2 = nc.dram_tensor("mk2", [SKtot, 2, 1], BF, kind="ExternalInput").ap()
    out = nc.dram_tensor("out", [2, DH + 1, SQtot], F32,
                         kind="ExternalOutput").ap()

    xq_r = xq.rearrange("(k p) s -> p k s", p=P)
    xk_r = xk.rearrange("(k p) s -> p k s", p=P)
    xv_r = xv.rearrange("(k p) s -> p k s", p=P)

    VW = 2 * (DH + 1)        # 130: [A num 64 | A mask | B num 64 | B mask]
    VWP = VW + DH - 1        # padded so lhsT for head B reads 128 cols

    with tile.TileContext(nc) as tc:
        with tc.tile_pool(name="const", bufs=1) as const, \
             tc.tile_pool(name="persist", bufs=1) as persist, \
             tc.tile_pool(name="xs", bufs=5) as xs, \
             tc.tile_pool(name="atp", bufs=2) as atp, \
             tc.tile_pool(name="otp", bufs=2) as otp, \
             tc.tile_pool(name="psq", bufs=2, space="PSUM") as psq, \
             tc.tile_pool(name="pss", bufs=2, space="PSUM") as pss:

            wq_sb = const.tile([P, KT, P], BF, tag="wq")
            wk_sb = const.tile([P, KT, P], BF, tag="wk")
            wv_sb = const.tile([P, KT, P], BF, tag="wv")
            nc.sync.dma_start(out=wq_sb, in_=wq.rearrange("(k p) e -> p k e", p=P))
            nc.sync.dma_start(out=wk_sb, in_=wk.rearrange("(k p) e -> p k e", p=P))
            nc.sync.dma_start(out=wv_sb, in_=wv.rearrange("(k p) e -> p k e", p=P))

            qt = persist.tile([P, SQtot], BF, tag="qt")
            kt = persist.tile([P, SKtot], BF, tag="kt")
            v_sb = persist.tile([P, SKTtot, VWP], BF, tag="v")
            # tail pad is read as lhsT columns for head B; zero it so
            # uninitialized SBUF never reaches PSUM
            nc.vector.memset(v_sb[:, :, VW:], 0.0)

            # ---------------- projection pieces (closures) ----------------
            def q_proj_piece(b, c0, n, dst, src_r, w_sb, off):
                def go():
                    xt = xs.tile([P, KT, 512], BF, tag="x")
                    nc.sync.dma_start(out=xt[:, :, :n],
                                      in_=src_r[:, :, off + c0:off + c0 + n])
                    ps = pss.tile([P, 512], F32, tag="acc")
                    for k in range(KT):
                        nc.tensor.matmul(ps[:, :n], w_sb[:, k, :], xt[:, k, :n],
                                         start=(k == 0), stop=(k == KT - 1))
                    nc.vector.tensor_copy(out=dst[:, off + c0:off + c0 + n],
                                          in_=ps[:, :n])
                return go

            def v_proj_piece(b, c0, n):
                def go():
                    xt = xs.tile([P, KT, 512], BF, tag="x")
                    nc.sync.dma_start(
                        out=xt[:, :, :n],
                        in_=xv_r[:, :, KOFF[b] + c0:KOFF[b] + c0 + n])
                    nt = n // P
                    ta = TOFF[b] + c0 // P
                    for st in range(nt):
                        ps = pss.tile([P, 512], F32, tag="acc")
                        for k in range(KT):
                            nc.tensor.matmul(
                                ps[:, :P], xt[:, k, P * st:P * (st + 1)],
                                wv_sb[:, k, :],
                                start=(k == 0), stop=(k == KT - 1))
                        vt = v_sb[:, ta + st, 0:VW].rearrange(
                            "p (g c) -> p g c", c=DH + 1)
                        nc.vector.tensor_copy(
                            out=vt[:, :, 0:DH],
                            in_=ps[:, :P].rearrange("p (g c) -> p g c", c=DH))
                        a0 = KOFF[b] + c0 + P * st
                        nc.sync.dma_start(out=vt[:, :, DH:DH + 1],
                                          in_=mk2[a0:a0 + P])
                return go

            def q_pieces(b):
                return [q_proj_piece(b, c0, n, qt, xq_r, wq_sb, QOFF[b])
                        for c0, n in _chunks(SQ[b])]

            def prelude_a(b):
                """What batch b's first QK group needs: kT + first q chunk."""
                ps_ = [q_proj_piece(b, c0, n, kt, xk_r, wk_sb, KOFF[b])
                       for c0, n in _chunks(SK[b])]
                ps_.append(q_pieces(b)[0])
                return ps_

            def prelude_b(b):
                """v projection -- only needed by PV, one chunk after QK."""
                return [v_proj_piece(b, c0, n) for c0, n in _chunks(SK[b])]

            # ---------------- attention ----------------
            def pv_pieces(b, c0, n, ats):
                """PV + evac + output-DMA closures for one finished chunk."""
                pieces = []

                def mk_pv(g, t0, t1, po_box):
                    def go():
                        if t0 == 0:
                            po_box[0] = pss.tile([P, 512], F32, tag="acc",
                                                 name=f"po_{b}_{c0}_{g}")
                        po = po_box[0]
                        for t in range(t0, t1):
                            nc.tensor.matmul(
                                po[:, :n],
                                v_sb[:, TOFF[b] + t, (DH + 1) * g:
                                     (DH + 1) * g + P],
                                ats[g][:, t, :n],
                                start=(t == 0), stop=(t == SKT[b] - 1))
                        if t1 == SKT[b]:
                            ot = otp.tile([DH + 1, 512], F32, tag="ot",
                                          name=f"ot_{b}_{c0}_{g}")
                            nc.vector.tensor_copy(out=ot[:, :n],
                                                  in_=po[0:DH + 1, :n])
                            # output DMA on the (idle) GpSimd queue so it
                            # never blocks input prefetch on the sync queue
                            nc.gpsimd.dma_start(
                                out=out[g, :, QOFF[b] + c0:QOFF[b] + c0 + n],
                                in_=ot[:, :n])
                    return go

                for g in range(2):
                    box = [None]
                    if SKT[b] > 6:
                        half = (SKT[b] + 1) // 2
                        pieces.append(mk_pv(g, 0, half, box))
                        pieces.append(mk_pv(g, half, SKT[b], box))
                    else:
                        pieces.append(mk_pv(g, 0, SKT[b], box))
                return pieces

            pend = []          # prev-chunk PV pieces (must emit next chunk)

            def emit_chunk(b, c0, n, fresh):
                """Emit one attention chunk; `fresh` (this batch's next q
                chunk + next batch's prelude ration) and the previous
                chunk's PV pieces are interleaved between QK groups."""
                nonlocal pend
                ats = (atp.tile([P, ATM, 512], BF, tag="ata",
                                name=f"ata_{b}_{c0}"),
                       atp.tile([P, ATM, 512], BF, tag="atb",
                                name=f"atb_{b}_{c0}"))
                groups = [(t0, min(GN, SKT[b] - t0))
                          for t0 in range(0, SKT[b], GN)]
                side = fresh + pend
                pend = []
                L = len(side)
                done = 0
                for gi, (t0, gn) in enumerate(groups):
                    pq = [psq.tile([P, GN, 512], F32, tag="qk",
                                   name=f"qk_{b}_{c0}_{t0}_{g}")
                          for g in range(2)]
                    for j in range(gn):
                        t = t0 + j
                        for g in range(2):
                            nc.tensor.matmul(
                                pq[g][:, j, :n],
                                kt[DH * g:DH * (g + 1),
                                   KOFF[b] + P * t:KOFF[b] + P * (t + 1)],
                                qt[DH * g:DH * (g + 1),
                                   QOFF[b] + c0:QOFF[b] + c0 + n],
                                start=True, stop=True)
                    for g in range(2):
                        nc.scalar.activation(
                            out=ats[g][:, t0:t0 + gn, :n],
                            in_=pq[g][:, 0:gn, :n],
                            func=Exp, scale=0.125)
                    upto = (L * (gi + 1)) // len(groups)
                    while done < upto:
                        side[done]()
                        done += 1
                while done < L:
                    side[done]()
                    done += 1
                pend = pv_pieces(b, c0, n, ats)

            # process batches LARGEST-first: the big batch's long exp phase
            # covers the later batches' input DMA, and only kT + q0 of the
            # first batch gate the start of attention (v streams during the
            # first chunk, the rest of q during later chunks)
            BORD = sorted(range(NB), key=lambda b: -SQT[b] * SKT[b])

            for piece in prelude_a(BORD[0]):
                piece()
            carry = prelude_b(BORD[0])
            for bi, b in enumerate(BORD):
                nxt = BORD[bi + 1] if bi + 1 < NB else None
                # carry = this batch's v pieces: MUST all emit during chunk 0
                # (PV of chunk 0 is emitted during chunk 1; a v-proj matmul
                # behind a PV matmul that needs it would deadlock the
                # in-order PE queue)
                must0 = carry
                filler = prelude_a(nxt) if nxt is not None else []
                carry = prelude_b(nxt) if nxt is not None else []
                qr = q_pieces(b)[1:]
                ch = _chunks(SQ[b])
                fdone = 0
                for ci, (c0, n) in enumerate(ch):
                    take = (len(filler) * (ci + 1)) // len(ch) - fdone
                    fresh = ([qr[ci]] if ci < len(qr) else []) \
                        + (must0 if ci == 0 else []) \
                        + filler[fdone:fdone + take]
                    fdone += take
                    emit_chunk(b, c0, n, fresh)
            for piece in pend:
                piece()

    nc.compile()
    return nc


def _get_program(SQT, SKT):
    key = (tuple(SQT), tuple(SKT))
    if key not in _PROG_CACHE:
        _PROG_CACHE[key] = _build_program(key[0], key[1])
    return _PROG_CACHE[key]


def _prep_inputs(Q_seq, K_seq, V_seq, WQ, WK, WV, Q_len, V_len):
    """Host-side shared prep: per-batch transposed bf16 activations and
    masks, concatenated along seq; returns (SQT, SKT, shared dict)."""
    BF = ml_dtypes.bfloat16
    B = Q_seq.shape[0]
    SQT = [max(1, math.ceil(int(Q_len[b]) / P)) for b in range(B)]
    SKT = [max(1, math.ceil(int(V_len[b]) / P)) for b in range(B)]
    SQ = [t * P for t in SQT]
    SK = [t * P for t in SKT]

    xq = np.concatenate(
        [np.ascontiguousarray(Q_seq[b, :SQ[b]].T) for b in range(B)],
        axis=1).astype(BF)
    xk = np.concatenate(
        [np.ascontiguousarray(K_seq[b, :SK[b]].T) for b in range(B)],
        axis=1).astype(BF)
    mks = [(np.arange(SK[b]) < int(V_len[b])) for b in range(B)]
    xv = np.concatenate(
        [np.ascontiguousarray((V_seq[b, :SK[b]] * mks[b][:, None]).T)
         for b in range(B)], axis=1).astype(BF)
    mk2 = np.concatenate(mks)[:, None, None].astype(BF)
    mk2 = np.repeat(mk2, 2, axis=1)
    return SQT, SKT, {"xq": xq, "xk": xk, "xv": xv, "mk2": mk2}


def kernel(Q_seq, K_seq, V_seq, WQ, WK, WV, Q_len, V_len):
    global _last_nc, _last_in_maps
    _ensure_paths()
    from concourse.bass_utils import run_bass_kernel_spmd

    Q_seq = np.asarray(Q_seq, dtype=np.float32)
    K_seq = np.asarray(K_seq, dtype=np.float32)
    V_seq = np.asarray(V_seq, dtype=np.float32)
    WQ = np.asarray(WQ, dtype=np.float32)
    WK = np.asarray(WK, dtype=np.float32)
    WV = np.asarray(WV, dtype=np.float32)
    Q_len = np.asarray(Q_len).reshape(-1)
    V_len = np.asarray(V_len).reshape(-1)

    B, S, _ = Q_seq.shape
    BF = ml_dtypes.bfloat16

    SQT, SKT, shared = _prep_inputs(Q_seq, K_seq, V_seq, WQ, WK, WV,
                                    Q_len, V_len)
    SQ = [t * P for t in SQT]
    QOFF = [sum(SQ[:b]) for b in range(B)]

    nc = _get_program(SQT, SKT)

    in_maps = []
    for c in range(NCORES):
        m = dict(shared)
        m["wq"] = np.ascontiguousarray(WQ[:, P * c:P * (c + 1)]).astype(BF)
        m["wk"] = np.ascontiguousarray(WK[:, P * c:P * (c + 1)]).astype(BF)
        m["wv"] = np.ascontiguousarray(WV[:, P * c:P * (c + 1)]).astype(BF)
        in_maps.append(m)

    res = run_bass_kernel_spmd(nc, in_maps, core_ids=list(range(NCORES)))
    _last_nc, _last_in_maps = nc, in_maps

    H = 16
    full = np.zeros((B, S, H * DH), dtype=np.float32)
    for c in range(NCORES):
        o = res.results[c]["out"]          # [2, 65, SQtot]
        for g in range(2):
            h = 2 * c + g
            num = o[g, :DH]                # [64, SQtot]
            den = o[g, DH:DH + 1]          # [1, SQtot]
            ot = num / den
            for b in range(B):
                ql = int(Q_len[b])
                sl = ot[:, QOFF[b]:QOFF[b] + SQ[b]]
                full[b, :SQ[b], DH * h:DH * (h + 1)] = sl.T
                full[b, ql:, DH * h:DH * (h + 1)] = 0.0
    return full
